# revision 60
# baseline (speedup 1.0000x reference)
"""Trainium2 Bass kernel for nn_MultiHeadAttention_52871047414119.

Reference (B=4, T=2048, D=512, H=8, DH=64, causal, eval):
    qkv = x @ w_qkv; per-head q,k,v
    out = concat_h(softmax(causal(q k^T / 8)) v) @ w_out

Sharding: 8 cores = 4 batches x 2 head-groups (4 heads each). Each core
returns the partial out-projection (bf16) for its head group; the host
upcasts and adds the two partials per batch.

v2 design (all matmul operands bf16; ~1 cycle/row everywhere):
  - host stages x^T (bf16) per core, so no on-device transposes of x.
  - PV is FLIPPED: stationary = 128-wide P chunk, moving = Vaug[kt,h]
    (64 V cols + ones column), accumulating per-q-subtile [128,4,65]
    PSUM tiles across k-tiles; ~half the PE streaming of natural PV.
    The ones column accumulates the softmax denominator per q row.
    PSUM zero-region semantics: only the tile's first matmul uses
    start=True; all other bytes lazily zero on first write.
  - normalize: reciprocal of the denom column (DVE, cheap), then
    per-subtile tensor_scalar multiply psum -> ON sbuf bf16.
  - O^T via PE transposes of ON 128-col chunks (2 heads packed), then
    out-proj from OT with moving wo (512 cols).
  - PE p-state warmup: dummy matmuls at t=0 ramp the clock to 2.4GHz
    during the input DMA; startup DMAs are ordered/split so the first
    Q/K chains (and so the first Exp) start as early as possible.
  - software-pipelined slot engine: the attention phase is a flat slot
    stream (head x k-pair); slot i+1's S matmuls are emitted DURING
    slot i, ahead of all deferred PE work, so the Exp stream on ScalarE
    is never gated by prep/PV/projection matmuls.
  - deferred work: per-window prep0 (own V chains + hp=1 Q/K chains,
    drained at in-window deadlines), prep (next window's hp=0 Q/K),
    aux (O transposes + projections, paced one per slot over the
    remaining program minus a tail reserve).
  - the last head's tail is pipelined: norms split per subtile-pair,
    its final k-pair split per-j, masks on DVE, copies/DMAs alternate
    ScalarE/DVE and the SP/Act DGE queues, so transposes / projections
    / output copies overlap the final Exps.

Cost model (TimelineSim): 105629 ns/core (v1 baseline 114752); measured
rel err vs fp32 reference: 5.6e-3 (bf16 operands, f32 accumulation).
Engine busy: Act (Exp) 79.9us — the bottleneck — PE 76.9us, DVE 48us,
Pool 18us. Remaining wall over the Act floor: DMA-bound startup ~7.5us
(4 serialized critical transfers + fixed DGE leads), window-0 is
PE-oversubscribed ~3us (its 8 Exp-slots cannot cover qc0+qc1 prep), and
the output tail ~5us (last-subtile norm/transpose/projection chain +
final DMA latency).
"""

import sys

for _p in ("/opt/trn_rl_repo",):
    if _p not in sys.path:
        sys.path.insert(0, _p)

import numpy as np

import concourse.bass as bass  # noqa: F401  (registers types)
import concourse.tile as tile
import concourse.mybir as mybir
from concourse import bacc
from concourse.masks import make_identity

F32 = mybir.dt.float32
BF16 = mybir.dt.bfloat16
AF = mybir.ActivationFunctionType
ALU = mybir.AluOpType

B, T, D, H, DH = 4, 2048, 512, 8, 64
NCORES = 8
HPC = 4          # heads per core
NTT = T // 128   # 16 row tiles
NDC = D // 128   # 4 contraction chunks
NQC = T // 512   # 4 q chunks
SCALE = 1.0 / np.sqrt(DH).item()
NWARM = 6        # PE clock warmup matmuls
POP_BUDGET = 2048   # max deferred PE cycles popped per slot
TOTAL_SLOTS = sum(8 * (qc + 1) for qc in range(NQC))


def emit_core_program(nc):
    xT = nc.dram_tensor("xT", [D, T], BF16, kind="ExternalInput").ap()
    wq = nc.dram_tensor("wq", [D, 256], BF16, kind="ExternalInput").ap()
    wk = nc.dram_tensor("wk", [D, 256], BF16, kind="ExternalInput").ap()
    wv = nc.dram_tensor("wv", [D, 256], BF16, kind="ExternalInput").ap()
    wo = nc.dram_tensor("wo", [256, D], BF16, kind="ExternalInput").ap()
    y = nc.dram_tensor("y", [T, D], BF16, kind="ExternalOutput").ap()

    xT_t = xT.rearrange("(dc p) t -> p dc t", p=128)    # [128,4,2048]
    wq_t = wq.rearrange("(dc p) m -> p dc m", p=128)    # [128,4,256]
    wk_t = wk.rearrange("(dc p) m -> p dc m", p=128)
    wv_t = wv.rearrange("(dc p) m -> p dc m", p=128)
    wo_t = wo.rearrange("(hp h2 dh) n -> (h2 dh) hp n", hp=2, h2=2)  # [128,2,512]
    y_t = y.rearrange("(tt p) d -> p tt d", p=128)

    with tile.TileContext(nc) as tc:
        with (
            tc.tile_pool(name="const", bufs=1) as constp,
            tc.tile_pool(name="wpool", bufs=1) as wpool,
            tc.tile_pool(name="big", bufs=1) as big,
            tc.tile_pool(name="ptp", bufs=8) as ptp,
            tc.tile_pool(name="smallp", bufs=4) as smallp,
            tc.tile_pool(name="yp", bufs=4) as yp,
            tc.tile_pool(name="psA", bufs=3, space="PSUM") as psA,
        ):
            ident_f32 = constp.tile([128, 128], F32)
            make_identity(nc, ident_f32)
            ident = constp.tile([128, 128], BF16)
            nc.vector.tensor_copy(ident, ident_f32)
            # lower-triangular causal mask tile (1 on/below diag) for the
            # tail's DVE-side masking
            tri = constp.tile([128, 128], BF16)
            nc.vector.memset(tri, 1.0)
            nc.gpsimd.affine_select(
                out=tri, in_=tri, pattern=[[1, 128]],
                compare_op=ALU.is_ge, fill=0.0, base=0,
                channel_multiplier=-1)

            # PE p-state warmup: no DMA deps, ramps the clock while
            # inputs stream in.
            warm = constp.tile([128, 512], BF16)
            nc.vector.memset(warm, 0.25)
            # dummy exp: hoists the 1.28us activation-table load to t~0
            # instead of right before the first real Exp
            actwarm = constp.tile([128, 1], F32)
            nc.scalar.activation(actwarm, warm[:, 0:1], AF.Exp, scale=1.0)
            for i in range(NWARM):
                wacc = psA.tile([128, 512], F32, tag="s", name=f"warm{i}")
                nc.tensor.matmul(wacc, warm[:, 0:128], warm,
                                 start=True, stop=True)

            xb = wpool.tile([128, NDC, T], BF16)        # x^T in sbuf
            wq_sb = wpool.tile([128, NDC, 256], BF16)
            wk_sb = wpool.tile([128, NDC, 256], BF16)
            wv_sb = wpool.tile([128, NDC, 256], BF16)
            wo_sb = wpool.tile([128, 2, 512], BF16)

            # DMA order: the hp=0 halves of wq/wk and x^T's first q-chunk
            # gate the first S matmul; minimize both the DMA count (HWDGE
            # setup serializes at 625ns each) and the critical bytes.
            nc.sync.dma_start(out=wq_sb, in_=wq_t)
            nc.scalar.dma_start(out=wk_sb, in_=wk_t)
            nc.sync.dma_start(out=xb[:, 0:2, 0:512], in_=xT_t[:, 0:2, 0:512])
            nc.scalar.dma_start(out=xb[:, 2:4, 0:512], in_=xT_t[:, 2:4, 0:512])
            nc.sync.dma_start(out=wv_sb, in_=wv_t)
            for qc in range(1, NQC):
                nc.sync.dma_start(out=xb[:, :, qc * 512:(qc + 1) * 512],
                                  in_=xT_t[:, :, qc * 512:(qc + 1) * 512])
            nc.sync.dma_start(out=wo_sb, in_=wo_t)

            QT = big.tile([128, 2, T], BF16)       # head-pair packed (dh of 2 heads)
            KT = big.tile([128, 2, T], BF16)
            Vaug = big.tile([128, NTT, HPC, DH + 1], BF16)  # V natural + ones col
            ON = big.tile([128, 2, T], BF16)       # normalized O, natural layout
            OT = big.tile([128, 2, T], BF16)       # O^T: [64*h2+dh, hp, t]

            nc.vector.memset(Vaug[:, :, :, 64:65], 1.0)  # denom col

            prep0_q = []     # (cost, fn, kind): current-window prep
            prep_q = []      # (cost, fn): next window's hp=0 Q/K chains
            aux_q = []       # (cost, fn): O transposes + projections
            norm_q = []      # eager: cheap on DVE, oacc-ring-critical
            norm_done = {}   # head index -> True once its norm was emitted
            pv_q = []        # pending PVs: run keep+1 slots after their S
            pv_keep = [1]
            window_left = [1]
            slots_left = [TOTAL_SLOTS]
            pace_acc = [0.0]
            aux_acc = [0.0]
            final_tail = [False]   # alternate Act/DVE copies for tail projs

            def qk_closure(w_sb, dst, hp, qc, use_act=False):
                def qk():
                    acc = psA.tile([128, 512], F32, tag="s",
                                   name=f"qkv{qc}_{hp}")
                    for dc in range(NDC):
                        nc.tensor.matmul(
                            acc,
                            w_sb[:, dc, hp * 128:(hp + 1) * 128],
                            xb[:, dc, qc * 512:(qc + 1) * 512],
                            start=(dc == 0),
                            stop=(dc == NDC - 1),
                        )
                    dst_ap = dst[:, hp, qc * 512:(qc + 1) * 512]
                    if use_act:
                        # early windows: DVE is the congested engine,
                        # ScalarE has slack
                        nc.scalar.copy(dst_ap, acc)
                    else:
                        nc.vector.tensor_copy(dst_ap, acc)
                return (2048, qk)

            def v_closure(tt, use_act=False):
                def vchain():
                    acc = psA.tile([128, 256], F32, tag="s", name=f"vacc{tt}")
                    for dc in range(NDC):
                        nc.tensor.matmul(
                            acc,
                            xb[:, dc, tt * 128:(tt + 1) * 128],
                            wv_sb[:, dc, :],
                            start=(dc == 0),
                            stop=(dc == NDC - 1),
                        )
                    dst_ap = Vaug[:, tt, :, 0:64]
                    src = acc.rearrange("p (h x) -> p h x", h=HPC)
                    if use_act:
                        nc.scalar.copy(dst_ap, src)
                    else:
                        nc.vector.tensor_copy(dst_ap, src)
                return (1024, vchain)

            def emit_transpose(hp, tt, use_act=False):
                otr = psA.tile([128, 128], BF16, tag="s", name=f"otr{hp}_{tt}")
                nc.tensor.transpose(otr, ON[:, hp, tt * 128:(tt + 1) * 128],
                                    ident)
                if use_act:
                    nc.scalar.copy(OT[:, hp, tt * 128:(tt + 1) * 128], otr)
                else:
                    nc.vector.tensor_copy(OT[:, hp, tt * 128:(tt + 1) * 128],
                                          otr)

            def emit_proj(tt):
                acc = psA.tile([128, 512], F32, tag="s", name=f"yacc{tt}")
                for hp_ in range(2):
                    nc.tensor.matmul(
                        acc,
                        OT[:, hp_, tt * 128:(tt + 1) * 128],
                        wo_sb[:, hp_, :],
                        start=(hp_ == 0),
                        stop=(hp_ == 1),
                    )
                ysb = yp.tile([128, 512], BF16, tag="ysb", name=f"ysb{tt}")
                if final_tail[0] and tt % 2 == 1:
                    nc.scalar.copy(ysb, acc)    # Act is idle in the tail
                    nc.sync.dma_start(out=y_t[:, tt, :], in_=ysb)
                elif final_tail[0]:
                    nc.vector.tensor_copy(ysb, acc)
                    # cross queues: DVE-copied tiles DMA via the Act DGE
                    nc.scalar.dma_start(out=y_t[:, tt, :], in_=ysb)
                else:
                    nc.vector.tensor_copy(ysb, acc)
                    nc.sync.dma_start(out=y_t[:, tt, :], in_=ysb)

            def emit_norm(oacc, hp, h2, qc, hidx, c0, c1, use_act=False):
                rc = smallp.tile([128, 4, 1], F32, tag="rc",
                                 name=f"rc{hidx}_{c0}", bufs=8)
                nc.vector.reciprocal(rc[:, c0:c1, :], oacc[:, c0:c1, 64:65])
                for c in range(c0, c1):
                    tt = 4 * qc + c
                    col = tt * 128 + h2 * 64
                    if use_act:
                        # normalize on ScalarE (idle in the tail): Copy
                        # activation with a per-partition scale vector
                        nc.scalar.activation(
                            ON[:, hp, col:col + 64], oacc[:, c, 0:64],
                            AF.Copy, scale=rc[:, c, :])
                    else:
                        nc.vector.tensor_scalar_mul(
                            ON[:, hp, col:col + 64],
                            oacc[:, c, 0:64],
                            rc[:, c, :],
                        )
                if c1 == 4:
                    norm_done[hidx] = True
                if h2 == 1:
                    # both heads of (qc, hp) normalized: O^T chunks ready
                    for c in range(c0, c1):
                        aux_q.append(
                            (160, lambda hp=hp, tt=4 * qc + c, ua=use_act:
                             emit_transpose(hp, tt, use_act=ua)))
                    if hp == 1:
                        for c in range(c0, c1):
                            aux_q.append(
                                (1100, lambda tt=4 * qc + c: emit_proj(tt)))

            def pop_one(aux=True):
                qs = (prep0_q, prep_q, aux_q) if aux else (prep0_q, prep_q)
                for q in qs:
                    if q:
                        cost, fn = q.pop(0)[:2]
                        fn()
                        return cost
                return 0

            def run_slot_prelude():
                while norm_q:
                    norm_q.pop(0)()
                # prep paced over this window; aux paced over the whole
                # remaining program (it has no deadline)
                wl = max(1, window_left[0])
                sl = max(1, slots_left[0] - 12)  # finish aux before the tail
                prep_supply = (sum(e[0] for e in prep0_q)
                               + sum(c for c, _ in prep_q))
                pace_acc[0] += prep_supply / wl
                aux_acc[0] += sum(c for c, _ in aux_q) / sl
                spent = 0
                while spent < min(pace_acc[0], POP_BUDGET) and (prep0_q
                                                               or prep_q):
                    got = pop_one(aux=False)
                    if not got:
                        break
                    spent += got
                pace_acc[0] = max(pace_acc[0] - spent, -1100.0)
                # aux: at most one item per slot, against its own budget
                if aux_q and aux_acc[0] >= aux_q[0][0]:
                    cost, fn = aux_q.pop(0)
                    fn()
                    aux_acc[0] -= cost
                window_left[0] -= 1
                slots_left[0] -= 1

            def pop_pv():
                fn, norm = pv_q.pop(0)
                fn()
                if norm is not None:
                    norm_q.append(norm)

            def run_prev_pv(keep=0):
                while len(pv_q) > keep:
                    pop_pv()

            def drain_prep0(kind=None):
                rest = []
                while prep0_q:
                    e = prep0_q.pop(0)
                    if kind is None or e[2] == kind:
                        e[1]()
                    else:
                        rest.append(e)
                prep0_q.extend(rest)

            def drain_all_prep():
                drain_prep0()
                while prep_q:
                    prep_q.pop(0)[1]()

            def emit_pv(pt, ktp, oacc, h, qc, js=(0, 1)):
                for j in js:
                    kt = 2 * ktp + j
                    c_min = max(0, kt - 4 * qc)
                    for c in range(c_min, 4):
                        # start only on the tile's very first matmul:
                        # start marks the whole 2KB psum zero-region
                        # pending, so later subtiles lazily zero on
                        # their first write.
                        nc.tensor.matmul(
                            oacc[:, c, :],
                            pt[:, j, c * 128:(c + 1) * 128],
                            Vaug[:, kt, h, :],
                            start=(kt == 0 and c == 0),
                            stop=(kt == 4 * qc + c),
                            skip_group_check=True,
                        )

            # ---- pipelined slot engine -------------------------------
            # The whole attention phase is a flat list of slots (head,
            # k-pair). S matmuls for slot i+1 are emitted DURING slot i,
            # ahead of all deferred PE work, so the Exp stream on ScalarE
            # is never gated by prep/PV/projection matmuls.

            heads = [(qc, hp, h2)
                     for qc in range(NQC)
                     for hp in range(2)
                     for h2 in range(2)]
            slots = []
            for hidx, (qc, hp, h2) in enumerate(heads):
                n_pairs = 2 * (qc + 1)
                is_last = hidx == len(heads) - 1
                n_full = n_pairs - 1 if is_last else n_pairs
                for ktp in range(n_full):
                    slots.append((hidx, qc, hp, h2, ktp))
            N_SLOTS = len(slots)

            oaccs = {}      # hidx -> oacc tile (alloc'd at first S)
            W = 512

            def window_prep0(qc):
                out = [v_closure(tt) + ("v",)
                       for tt in range(4 * qc, 4 * qc + 4)]
                out.append(qk_closure(wq_sb, QT, 1, qc) + ("qk",))
                out.append(qk_closure(wk_sb, KT, 1, qc) + ("qk",))
                return out

            cur_qc = [-1]

            def pre_slot_gates(i):
                """Drains/allocs that must precede slot i's S matmuls."""
                hidx, qc, hp, h2, ktp = slots[i]
                if ktp == 0 and hp == 1 and h2 == 0:
                    drain_prep0(kind="qk")
                if qc != cur_qc[0]:
                    while prep_q:
                        prep_q.pop(0)[1]()
                    prep0_q.extend(window_prep0(qc))
                    if qc + 1 < NQC:
                        ua = False
                        prep_q.append(qk_closure(wq_sb, QT, 0, qc + 1,
                                                 use_act=ua))
                        prep_q.append(qk_closure(wk_sb, KT, 0, qc + 1,
                                                 use_act=ua))
                    cur_qc[0] = qc
                    window_left[0] = 8 * (qc + 1)
                    pv_keep[0] = 2 if qc in (1, 2) else 1
                if ktp == 0:
                    # oacc ring (2 deep): norm of the head 2 back must
                    # have been emitted before reuse
                    _spin = 0
                    while hidx >= 2 and not norm_done.get(hidx - 2, False):
                        _spin += 1
                        if _spin > 10000:
                            raise RuntimeError(f"ensure spin: {hidx}")
                        # flush PVs/norms first -- popping whole prep
                        # queues here would burst the PE right when the
                        # next window's Exp stream needs its S matmuls
                        if norm_q:
                            norm_q.pop(0)()
                        elif pv_q:
                            pop_pv()
                        else:
                            pop_one()
                    oaccs[hidx] = psA.tile([128, 4, DH + 1], F32, tag="o",
                                           bufs=2, name=f"oacc{hidx}")

            def emit_S(i):
                hidx, qc, hp, h2, ktp = slots[i]
                hb = 64 * h2
                q0 = qc * 512
                lo = max(0, min(2 * ktp * 128 - q0, W))
                s = psA.tile([128, 2, 512], F32, tag="s",
                             name=f"s{hidx}_{ktp}")
                for j in range(2):
                    kt = 2 * ktp + j
                    # S written from pair-level lo: Exp reads [lo:W] for
                    # both j, and CoreSim rejects never-written PSUM reads.
                    nc.tensor.matmul(
                        s[:, j, lo:W],
                        KT[hb:hb + 64, hp, kt * 128:(kt + 1) * 128],
                        QT[hb:hb + 64, hp, q0 + lo:q0 + W],
                        start=True,
                        stop=True,
                    )
                return s, lo

            def emit_exp_mask(i, s, lo):
                hidx, qc, hp, h2, ktp = slots[i]
                q0 = qc * 512
                is_last = hidx == len(heads) - 1
                pt = ptp.tile([128, 2, 512], BF16, tag="pt",
                              name=f"pt{hidx}_{ktp}")
                nc.scalar.activation(pt[:, :, lo:W], s[:, :, lo:W],
                                     AF.Exp, scale=SCALE)
                for j in range(2):
                    kt = 2 * ktp + j
                    off = kt * 128 - q0
                    if 0 <= off < W:
                        w_end = min(off + 128, W)
                        if is_last and ktp >= 2 * (qc + 1) - 2:
                            # DVE mask: keeps Pool launch latency out of
                            # the tail's critical chain
                            nc.vector.tensor_mul(
                                pt[:, j, off:w_end],
                                pt[:, j, off:w_end],
                                tri[:, 0:w_end - off],
                            )
                        else:
                            nc.gpsimd.affine_select(
                                out=pt[:, j, off:w_end],
                                in_=pt[:, j, off:w_end],
                                pattern=[[1, w_end - off]],
                                compare_op=ALU.is_ge,
                                fill=0.0,
                                base=0,
                                channel_multiplier=-1,
                            )
                return pt

            # fast start: the hp=0 Q/K chains inline, then slot 0's S.
            qk_closure(wq_sb, QT, 0, 0)[1]()
            qk_closure(wk_sb, KT, 0, 0)[1]()
            pre_slot_gates(0)
            pend = emit_S(0)

            for i in range(N_SLOTS):
                hidx, qc, hp, h2, ktp = slots[i]
                h = 2 * hp + h2
                is_last = hidx == len(heads) - 1
                n_pairs = 2 * (qc + 1)
                oacc = oaccs[hidx]
                s, lo = pend
                pt = emit_exp_mask(i, s, lo)
                # S for the NEXT slot goes ahead of all deferred PE work
                if i + 1 < N_SLOTS:
                    pre_slot_gates(i + 1)
                    pend = emit_S(i + 1)
                if hidx % 4 == 0 and ktp == max(1, 2 * qc + 1):
                    # V tiles for this window's diagonal k-tiles are
                    # needed by the PVs a couple of slots from now
                    drain_prep0(kind="v")
                run_slot_prelude()
                keep = 0 if (is_last and ktp >= n_pairs - 2) else pv_keep[0]
                run_prev_pv(keep=keep)
                while norm_q:
                    norm_q.pop(0)()

                norm = None
                if ktp == n_pairs - 1:
                    norm = (lambda oacc=oacc, hp=hp, h2=h2, qc=qc,
                            hidx=hidx: emit_norm(oacc, hp, h2, qc,
                                                 hidx, 0, 4))
                elif is_last and ktp == n_pairs - 2:
                    # early norm of subtiles 0,1: overlaps the tail;
                    # alternate DVE/Act so neither serializes the chains
                    def norm(oacc=oacc, hp=hp, h2=h2, qc=qc, hidx=hidx):
                        emit_norm(oacc, hp, h2, qc, hidx, 0, 1)
                        emit_norm(oacc, hp, h2, qc, hidx, 1, 2,
                                  use_act=True)
                pv_q.append(
                    (lambda pt=pt, ktp=ktp, oacc=oacc, h=h, qc=qc:
                     emit_pv(pt, ktp, oacc, h, qc), norm))

            # last head's final k-pair, split per-j: norms/transposes/
            # projections pipeline into the Exp gaps. Both j's S/Exp are
            # emitted first so ScalarE stays packed.
            hidx, (qc, hp, h2) = len(heads) - 1, heads[-1]
            h = 2 * hp + h2
            hb = 64 * h2
            q0 = qc * 512
            oacc = oaccs[hidx]
            ktp = 2 * (qc + 1) - 1
            pts = []
            for j in range(2):
                kt = 2 * ktp + j
                lo_j = kt * 128 - q0
                s = psA.tile([128, 2, 512], F32, tag="s", name=f"sL{j}")
                nc.tensor.matmul(
                    s[:, j, lo_j:W],
                    KT[hb:hb + 64, hp, kt * 128:(kt + 1) * 128],
                    QT[hb:hb + 64, hp, q0 + lo_j:q0 + W],
                    start=True, stop=True,
                )
                pt = ptp.tile([128, 2, 512], BF16, tag="pt", name=f"ptL{j}")
                nc.scalar.activation(pt[:, j, lo_j:W], s[:, j, lo_j:W],
                                     AF.Exp, scale=SCALE)
                nc.vector.tensor_mul(
                    pt[:, j, lo_j:lo_j + 128],
                    pt[:, j, lo_j:lo_j + 128],
                    tri[:, 0:128],
                )
                pts.append((pt, kt))
            run_prev_pv(keep=0)
            while norm_q:
                norm_q.pop(0)()
            final_tail[0] = True
            # both PV blocks + norms FIRST: the c3-critical chain must not
            # queue behind the earlier tiles' transposes/projections
            for j, (pt, kt) in enumerate(pts):
                for c in range(2 + j, 4):
                    nc.tensor.matmul(
                        oacc[:, c, :],
                        pt[:, j, c * 128:(c + 1) * 128],
                        Vaug[:, kt, h, :],
                        start=False, stop=(c == 2 + j),
                        skip_group_check=True,
                    )
                emit_norm(oacc, hp, h2, qc, hidx, 2 + j, 3 + j,
                          use_act=(j == 1))
            while aux_q:
                aux_q.pop(0)[1]()

            # drain anything left
            drain_all_prep()
            run_prev_pv()
            while norm_q or aux_q:
                if norm_q:
                    norm_q.pop(0)()
                else:
                    aux_q.pop(0)[1]()

    return nc


_NC_CACHE = None


def get_nc():
    global _NC_CACHE
    if _NC_CACHE is None:
        nc = bacc.Bacc("TRN2", target_bir_lowering=False, debug=False,
                       num_devices=NCORES)
        emit_core_program(nc)
        nc.compile()
        _NC_CACHE = nc
    return _NC_CACHE


def make_in_maps(x, w_qkv, w_out):
    import ml_dtypes
    BF = ml_dtypes.bfloat16
    x = np.asarray(x, dtype=np.float32)
    w_qkv = np.asarray(w_qkv, dtype=np.float32)
    w_out = np.asarray(w_out, dtype=np.float32)
    in_maps = []
    for c in range(NCORES):
        b, g = c // 2, c % 2
        lo = 256 * g
        in_maps.append({
            "xT": np.ascontiguousarray(x[b].T.astype(BF)),
            "wq": np.ascontiguousarray(w_qkv[:, lo:lo + 256].astype(BF)),
            "wk": np.ascontiguousarray(w_qkv[:, 512 + lo:512 + lo + 256].astype(BF)),
            "wv": np.ascontiguousarray(w_qkv[:, 1024 + lo:1024 + lo + 256].astype(BF)),
            "wo": np.ascontiguousarray(w_out[lo:lo + 256, :].astype(BF)),
        })
    return in_maps


def assemble_output(results):
    out = np.empty((B, T, D), dtype=np.float32)
    for b in range(B):
        out[b] = (np.asarray(results[2 * b]["y"], dtype=np.float32)
                  + np.asarray(results[2 * b + 1]["y"], dtype=np.float32))
    return out


def kernel(x, w_qkv, w_out):
    from concourse.bass_utils import run_bass_kernel_spmd

    nc = get_nc()
    in_maps = make_in_maps(x, w_qkv, w_out)
    res = run_bass_kernel_spmd(nc, in_maps, list(range(NCORES))).results
    return assemble_output(res)


# revision 61
# speedup vs baseline: 1.0006x; 1.0006x over previous
"""Trainium2 Bass kernel for nn_MultiHeadAttention_52871047414119.

Reference (B=4, T=2048, D=512, H=8, DH=64, causal, eval):
    qkv = x @ w_qkv; per-head q,k,v
    out = concat_h(softmax(causal(q k^T / 8)) v) @ w_out

Sharding: 8 cores = 4 batches x 2 head-groups (4 heads each). Each core
returns the partial out-projection (bf16) for its head group; the host
upcasts and adds the two partials per batch.

v2 design (all matmul operands bf16; ~1 cycle/row everywhere):
  - host stages x^T (bf16) per core, so no on-device transposes of x.
  - PV is FLIPPED: stationary = 128-wide P chunk, moving = Vaug[kt,h]
    (64 V cols + ones column), accumulating per-q-subtile [128,4,65]
    PSUM tiles across k-tiles; ~half the PE streaming of natural PV.
    The ones column accumulates the softmax denominator per q row.
    PSUM zero-region semantics: only the tile's first matmul uses
    start=True; all other bytes lazily zero on first write.
  - normalize: reciprocal of the denom column (DVE, cheap), then
    per-subtile tensor_scalar multiply psum -> ON sbuf bf16.
  - O^T via PE transposes of ON 128-col chunks (2 heads packed), then
    out-proj from OT with moving wo (512 cols).
  - PE p-state warmup: dummy matmuls at t=0 ramp the clock to 2.4GHz
    during the input DMA; startup DMAs are ordered/split so the first
    Q/K chains (and so the first Exp) start as early as possible.
  - software-pipelined slot engine: the attention phase is a flat slot
    stream (head x k-pair); slot i+1's S matmuls are emitted DURING
    slot i, ahead of all deferred PE work, so the Exp stream on ScalarE
    is never gated by prep/PV/projection matmuls.
  - deferred work: per-window prep0 (own V chains + hp=1 Q/K chains,
    drained at in-window deadlines), prep (next window's hp=0 Q/K),
    aux (O transposes + projections, paced one per slot over the
    remaining program minus a tail reserve).
  - the last head's tail is pipelined: norms split per subtile-pair,
    its final k-pair split per-j, masks on DVE, copies/DMAs alternate
    ScalarE/DVE and the SP/Act DGE queues, so transposes / projections
    / output copies overlap the final Exps.

Cost model (TimelineSim): 105570 ns/core (v1 baseline 114752); measured
rel err vs fp32 reference: 5.6e-3 (bf16 operands, f32 accumulation).
Engine busy: Act (Exp) 79.9us — the bottleneck — PE 76.9us, DVE 48us,
Pool 18us. Remaining wall over the Act floor: DMA-bound startup ~7.5us
(4 serialized critical transfers + fixed DGE leads), window-0 is
PE-oversubscribed ~3us (its 8 Exp-slots cannot cover qc0+qc1 prep), and
the output tail ~5us (last-subtile norm/transpose/projection chain +
final DMA latency).
"""

import sys

for _p in ("/opt/trn_rl_repo",):
    if _p not in sys.path:
        sys.path.insert(0, _p)

import numpy as np

import concourse.bass as bass  # noqa: F401  (registers types)
import concourse.tile as tile
import concourse.mybir as mybir
from concourse import bacc
from concourse.masks import make_identity

F32 = mybir.dt.float32
BF16 = mybir.dt.bfloat16
AF = mybir.ActivationFunctionType
ALU = mybir.AluOpType

B, T, D, H, DH = 4, 2048, 512, 8, 64
NCORES = 8
HPC = 4          # heads per core
NTT = T // 128   # 16 row tiles
NDC = D // 128   # 4 contraction chunks
NQC = T // 512   # 4 q chunks
SCALE = 1.0 / np.sqrt(DH).item()
NWARM = 6        # PE clock warmup matmuls
POP_BUDGET = 2048   # max deferred PE cycles popped per slot
TOTAL_SLOTS = sum(8 * (qc + 1) for qc in range(NQC))


def emit_core_program(nc):
    xT = nc.dram_tensor("xT", [D, T], BF16, kind="ExternalInput").ap()
    wq = nc.dram_tensor("wq", [D, 256], BF16, kind="ExternalInput").ap()
    wk = nc.dram_tensor("wk", [D, 256], BF16, kind="ExternalInput").ap()
    wv = nc.dram_tensor("wv", [D, 256], BF16, kind="ExternalInput").ap()
    wo = nc.dram_tensor("wo", [256, D], BF16, kind="ExternalInput").ap()
    y = nc.dram_tensor("y", [T, D], BF16, kind="ExternalOutput").ap()

    xT_t = xT.rearrange("(dc p) t -> p dc t", p=128)    # [128,4,2048]
    wq_t = wq.rearrange("(dc p) m -> p dc m", p=128)    # [128,4,256]
    wk_t = wk.rearrange("(dc p) m -> p dc m", p=128)
    wv_t = wv.rearrange("(dc p) m -> p dc m", p=128)
    wo_t = wo.rearrange("(hp h2 dh) n -> (h2 dh) hp n", hp=2, h2=2)  # [128,2,512]
    y_t = y.rearrange("(tt p) d -> p tt d", p=128)

    with tile.TileContext(nc) as tc:
        with (
            tc.tile_pool(name="const", bufs=1) as constp,
            tc.tile_pool(name="wpool", bufs=1) as wpool,
            tc.tile_pool(name="big", bufs=1) as big,
            tc.tile_pool(name="ptp", bufs=10) as ptp,
            tc.tile_pool(name="smallp", bufs=4) as smallp,
            tc.tile_pool(name="yp", bufs=4) as yp,
            tc.tile_pool(name="psA", bufs=3, space="PSUM") as psA,
        ):
            ident_f32 = constp.tile([128, 128], F32)
            make_identity(nc, ident_f32)
            ident = constp.tile([128, 128], BF16)
            nc.vector.tensor_copy(ident, ident_f32)
            # lower-triangular causal mask tile (1 on/below diag) for the
            # tail's DVE-side masking
            tri = constp.tile([128, 128], BF16)
            nc.vector.memset(tri, 1.0)
            nc.gpsimd.affine_select(
                out=tri, in_=tri, pattern=[[1, 128]],
                compare_op=ALU.is_ge, fill=0.0, base=0,
                channel_multiplier=-1)

            # PE p-state warmup: no DMA deps, ramps the clock while
            # inputs stream in.
            warm = constp.tile([128, 512], BF16)
            nc.vector.memset(warm, 0.25)
            # dummy exp: hoists the 1.28us activation-table load to t~0
            # instead of right before the first real Exp
            actwarm = constp.tile([128, 1], F32)
            nc.scalar.activation(actwarm, warm[:, 0:1], AF.Exp, scale=1.0)
            for i in range(NWARM):
                wacc = psA.tile([128, 512], F32, tag="s", name=f"warm{i}")
                nc.tensor.matmul(wacc, warm[:, 0:128], warm,
                                 start=True, stop=True)

            xb = wpool.tile([128, NDC, T], BF16)        # x^T in sbuf
            wq_sb = wpool.tile([128, NDC, 256], BF16)
            wk_sb = wpool.tile([128, NDC, 256], BF16)
            wv_sb = wpool.tile([128, NDC, 256], BF16)
            wo_sb = wpool.tile([128, 2, 512], BF16)

            # DMA order: the hp=0 halves of wq/wk and x^T's first q-chunk
            # gate the first S matmul; minimize both the DMA count (HWDGE
            # setup serializes at 625ns each) and the critical bytes.
            nc.sync.dma_start(out=wq_sb, in_=wq_t)
            nc.scalar.dma_start(out=wk_sb, in_=wk_t)
            nc.sync.dma_start(out=xb[:, 0:2, 0:512], in_=xT_t[:, 0:2, 0:512])
            nc.scalar.dma_start(out=xb[:, 2:4, 0:512], in_=xT_t[:, 2:4, 0:512])
            nc.sync.dma_start(out=wv_sb, in_=wv_t)
            for qc in range(1, NQC):
                nc.sync.dma_start(out=xb[:, :, qc * 512:(qc + 1) * 512],
                                  in_=xT_t[:, :, qc * 512:(qc + 1) * 512])
            nc.sync.dma_start(out=wo_sb, in_=wo_t)

            QT = big.tile([128, 2, T], BF16)       # head-pair packed (dh of 2 heads)
            KT = big.tile([128, 2, T], BF16)
            Vaug = big.tile([128, NTT, HPC, DH + 1], BF16)  # V natural + ones col
            ON = big.tile([128, 2, T], BF16)       # normalized O, natural layout
            OT = big.tile([128, 2, T], BF16)       # O^T: [64*h2+dh, hp, t]

            nc.vector.memset(Vaug[:, :, :, 64:65], 1.0)  # denom col

            prep0_q = []     # (cost, fn, kind): current-window prep
            prep_q = []      # (cost, fn): next window's hp=0 Q/K chains
            aux_q = []       # (cost, fn): O transposes + projections
            norm_q = []      # eager: cheap on DVE, oacc-ring-critical
            norm_done = {}   # head index -> True once its norm was emitted
            pv_q = []        # pending PVs: run keep+1 slots after their S
            pv_keep = [1]
            window_left = [1]
            slots_left = [TOTAL_SLOTS]
            pace_acc = [0.0]
            aux_acc = [0.0]
            final_tail = [False]   # alternate Act/DVE copies for tail projs

            def qk_closure(w_sb, dst, hp, qc, use_act=False):
                def qk():
                    acc = psA.tile([128, 512], F32, tag="s",
                                   name=f"qkv{qc}_{hp}")
                    for dc in range(NDC):
                        nc.tensor.matmul(
                            acc,
                            w_sb[:, dc, hp * 128:(hp + 1) * 128],
                            xb[:, dc, qc * 512:(qc + 1) * 512],
                            start=(dc == 0),
                            stop=(dc == NDC - 1),
                        )
                    dst_ap = dst[:, hp, qc * 512:(qc + 1) * 512]
                    if use_act:
                        # early windows: DVE is the congested engine,
                        # ScalarE has slack
                        nc.scalar.copy(dst_ap, acc)
                    else:
                        nc.vector.tensor_copy(dst_ap, acc)
                return (2048, qk)

            def v_closure(tt, use_act=False):
                def vchain():
                    acc = psA.tile([128, 256], F32, tag="s", name=f"vacc{tt}")
                    for dc in range(NDC):
                        nc.tensor.matmul(
                            acc,
                            xb[:, dc, tt * 128:(tt + 1) * 128],
                            wv_sb[:, dc, :],
                            start=(dc == 0),
                            stop=(dc == NDC - 1),
                        )
                    dst_ap = Vaug[:, tt, :, 0:64]
                    src = acc.rearrange("p (h x) -> p h x", h=HPC)
                    if use_act:
                        nc.scalar.copy(dst_ap, src)
                    else:
                        nc.vector.tensor_copy(dst_ap, src)
                return (1024, vchain)

            def emit_transpose(hp, tt, use_act=False):
                otr = psA.tile([128, 128], BF16, tag="s", name=f"otr{hp}_{tt}")
                nc.tensor.transpose(otr, ON[:, hp, tt * 128:(tt + 1) * 128],
                                    ident)
                if use_act:
                    nc.scalar.copy(OT[:, hp, tt * 128:(tt + 1) * 128], otr)
                else:
                    nc.vector.tensor_copy(OT[:, hp, tt * 128:(tt + 1) * 128],
                                          otr)

            def emit_proj(tt):
                acc = psA.tile([128, 512], F32, tag="s", name=f"yacc{tt}")
                for hp_ in range(2):
                    nc.tensor.matmul(
                        acc,
                        OT[:, hp_, tt * 128:(tt + 1) * 128],
                        wo_sb[:, hp_, :],
                        start=(hp_ == 0),
                        stop=(hp_ == 1),
                    )
                ysb = yp.tile([128, 512], BF16, tag="ysb", name=f"ysb{tt}")
                if final_tail[0] and tt % 2 == 1:
                    nc.scalar.copy(ysb, acc)    # Act is idle in the tail
                    nc.sync.dma_start(out=y_t[:, tt, :], in_=ysb)
                elif final_tail[0]:
                    nc.vector.tensor_copy(ysb, acc)
                    # cross queues: DVE-copied tiles DMA via the Act DGE
                    nc.scalar.dma_start(out=y_t[:, tt, :], in_=ysb)
                else:
                    nc.vector.tensor_copy(ysb, acc)
                    nc.sync.dma_start(out=y_t[:, tt, :], in_=ysb)

            def emit_norm(oacc, hp, h2, qc, hidx, c0, c1, use_act=False):
                rc = smallp.tile([128, 4, 1], F32, tag="rc",
                                 name=f"rc{hidx}_{c0}", bufs=8)
                nc.vector.reciprocal(rc[:, c0:c1, :], oacc[:, c0:c1, 64:65])
                for c in range(c0, c1):
                    tt = 4 * qc + c
                    col = tt * 128 + h2 * 64
                    if use_act:
                        # normalize on ScalarE (idle in the tail): Copy
                        # activation with a per-partition scale vector
                        nc.scalar.activation(
                            ON[:, hp, col:col + 64], oacc[:, c, 0:64],
                            AF.Copy, scale=rc[:, c, :])
                    else:
                        nc.vector.tensor_scalar_mul(
                            ON[:, hp, col:col + 64],
                            oacc[:, c, 0:64],
                            rc[:, c, :],
                        )
                if c1 == 4:
                    norm_done[hidx] = True
                if h2 == 1:
                    # both heads of (qc, hp) normalized: O^T chunks ready
                    for c in range(c0, c1):
                        aux_q.append(
                            (160, lambda hp=hp, tt=4 * qc + c, ua=use_act:
                             emit_transpose(hp, tt, use_act=ua)))
                    if hp == 1:
                        for c in range(c0, c1):
                            aux_q.append(
                                (1100, lambda tt=4 * qc + c: emit_proj(tt)))

            def pop_one(aux=True):
                qs = (prep0_q, prep_q, aux_q) if aux else (prep0_q, prep_q)
                for q in qs:
                    if q:
                        cost, fn = q.pop(0)[:2]
                        fn()
                        return cost
                return 0

            def run_slot_prelude():
                while norm_q:
                    norm_q.pop(0)()
                # prep paced over this window; aux paced over the whole
                # remaining program (it has no deadline)
                wl = max(1, window_left[0])
                sl = max(1, slots_left[0] - 12)  # finish aux before the tail
                prep_supply = (sum(e[0] for e in prep0_q)
                               + sum(c for c, _ in prep_q))
                pace_acc[0] += prep_supply / wl
                aux_acc[0] += sum(c for c, _ in aux_q) / sl
                spent = 0
                while spent < min(pace_acc[0], POP_BUDGET) and (prep0_q
                                                               or prep_q):
                    got = pop_one(aux=False)
                    if not got:
                        break
                    spent += got
                pace_acc[0] = max(pace_acc[0] - spent, -1100.0)
                # aux: at most one item per slot, against its own budget
                if aux_q and aux_acc[0] >= aux_q[0][0]:
                    cost, fn = aux_q.pop(0)
                    fn()
                    aux_acc[0] -= cost
                window_left[0] -= 1
                slots_left[0] -= 1

            def pop_pv():
                fn, norm = pv_q.pop(0)
                fn()
                if norm is not None:
                    norm_q.append(norm)

            def run_prev_pv(keep=0):
                while len(pv_q) > keep:
                    pop_pv()

            def drain_prep0(kind=None):
                rest = []
                while prep0_q:
                    e = prep0_q.pop(0)
                    if kind is None or e[2] == kind:
                        e[1]()
                    else:
                        rest.append(e)
                prep0_q.extend(rest)

            def drain_all_prep():
                drain_prep0()
                while prep_q:
                    prep_q.pop(0)[1]()

            def emit_pv(pt, ktp, oacc, h, qc, js=(0, 1)):
                for j in js:
                    kt = 2 * ktp + j
                    c_min = max(0, kt - 4 * qc)
                    for c in range(c_min, 4):
                        # start only on the tile's very first matmul:
                        # start marks the whole 2KB psum zero-region
                        # pending, so later subtiles lazily zero on
                        # their first write.
                        nc.tensor.matmul(
                            oacc[:, c, :],
                            pt[:, j, c * 128:(c + 1) * 128],
                            Vaug[:, kt, h, :],
                            start=(kt == 0 and c == 0),
                            stop=(kt == 4 * qc + c),
                            skip_group_check=True,
                        )

            # ---- pipelined slot engine -------------------------------
            # The whole attention phase is a flat list of slots (head,
            # k-pair). S matmuls for slot i+1 are emitted DURING slot i,
            # ahead of all deferred PE work, so the Exp stream on ScalarE
            # is never gated by prep/PV/projection matmuls.

            heads = [(qc, hp, h2)
                     for qc in range(NQC)
                     for hp in range(2)
                     for h2 in range(2)]
            slots = []
            for hidx, (qc, hp, h2) in enumerate(heads):
                n_pairs = 2 * (qc + 1)
                is_last = hidx == len(heads) - 1
                n_full = n_pairs - 1 if is_last else n_pairs
                for ktp in range(n_full):
                    slots.append((hidx, qc, hp, h2, ktp))
            N_SLOTS = len(slots)

            oaccs = {}      # hidx -> oacc tile (alloc'd at first S)
            W = 512

            def window_prep0(qc):
                out = [v_closure(tt) + ("v",)
                       for tt in range(4 * qc, 4 * qc + 4)]
                out.append(qk_closure(wq_sb, QT, 1, qc) + ("qk",))
                out.append(qk_closure(wk_sb, KT, 1, qc) + ("qk",))
                return out

            cur_qc = [-1]

            def pre_slot_gates(i):
                """Drains/allocs that must precede slot i's S matmuls."""
                hidx, qc, hp, h2, ktp = slots[i]
                if ktp == 0 and hp == 1 and h2 == 0:
                    drain_prep0(kind="qk")
                if qc != cur_qc[0]:
                    while prep_q:
                        prep_q.pop(0)[1]()
                    prep0_q.extend(window_prep0(qc))
                    if qc + 1 < NQC:
                        ua = False
                        prep_q.append(qk_closure(wq_sb, QT, 0, qc + 1,
                                                 use_act=ua))
                        prep_q.append(qk_closure(wk_sb, KT, 0, qc + 1,
                                                 use_act=ua))
                    cur_qc[0] = qc
                    window_left[0] = 8 * (qc + 1)
                    pv_keep[0] = 2 if qc in (1, 2) else 1
                if ktp == 0:
                    # oacc ring (2 deep): norm of the head 2 back must
                    # have been emitted before reuse
                    _spin = 0
                    while hidx >= 2 and not norm_done.get(hidx - 2, False):
                        _spin += 1
                        if _spin > 10000:
                            raise RuntimeError(f"ensure spin: {hidx}")
                        # flush PVs/norms first -- popping whole prep
                        # queues here would burst the PE right when the
                        # next window's Exp stream needs its S matmuls
                        if norm_q:
                            norm_q.pop(0)()
                        elif pv_q:
                            pop_pv()
                        else:
                            pop_one()
                    oaccs[hidx] = psA.tile([128, 4, DH + 1], F32, tag="o",
                                           bufs=2, name=f"oacc{hidx}")

            def emit_S(i):
                hidx, qc, hp, h2, ktp = slots[i]
                hb = 64 * h2
                q0 = qc * 512
                lo = max(0, min(2 * ktp * 128 - q0, W))
                s = psA.tile([128, 2, 512], F32, tag="s",
                             name=f"s{hidx}_{ktp}")
                for j in range(2):
                    kt = 2 * ktp + j
                    # S written from pair-level lo: Exp reads [lo:W] for
                    # both j, and CoreSim rejects never-written PSUM reads.
                    nc.tensor.matmul(
                        s[:, j, lo:W],
                        KT[hb:hb + 64, hp, kt * 128:(kt + 1) * 128],
                        QT[hb:hb + 64, hp, q0 + lo:q0 + W],
                        start=True,
                        stop=True,
                    )
                return s, lo

            def emit_exp_mask(i, s, lo):
                hidx, qc, hp, h2, ktp = slots[i]
                q0 = qc * 512
                is_last = hidx == len(heads) - 1
                pt = ptp.tile([128, 2, 512], BF16, tag="pt",
                              name=f"pt{hidx}_{ktp}")
                nc.scalar.activation(pt[:, :, lo:W], s[:, :, lo:W],
                                     AF.Exp, scale=SCALE)
                for j in range(2):
                    kt = 2 * ktp + j
                    off = kt * 128 - q0
                    if 0 <= off < W:
                        w_end = min(off + 128, W)
                        if is_last and ktp >= 2 * (qc + 1) - 2:
                            # DVE mask: keeps Pool launch latency out of
                            # the tail's critical chain
                            nc.vector.tensor_mul(
                                pt[:, j, off:w_end],
                                pt[:, j, off:w_end],
                                tri[:, 0:w_end - off],
                            )
                        else:
                            nc.gpsimd.affine_select(
                                out=pt[:, j, off:w_end],
                                in_=pt[:, j, off:w_end],
                                pattern=[[1, w_end - off]],
                                compare_op=ALU.is_ge,
                                fill=0.0,
                                base=0,
                                channel_multiplier=-1,
                            )
                return pt

            # fast start: the hp=0 Q/K chains inline, then slot 0's S.
            qk_closure(wq_sb, QT, 0, 0)[1]()
            qk_closure(wk_sb, KT, 0, 0)[1]()
            pre_slot_gates(0)
            pend = emit_S(0)

            for i in range(N_SLOTS):
                hidx, qc, hp, h2, ktp = slots[i]
                h = 2 * hp + h2
                is_last = hidx == len(heads) - 1
                n_pairs = 2 * (qc + 1)
                oacc = oaccs[hidx]
                s, lo = pend
                pt = emit_exp_mask(i, s, lo)
                # S for the NEXT slot goes ahead of all deferred PE work
                if i + 1 < N_SLOTS:
                    pre_slot_gates(i + 1)
                    pend = emit_S(i + 1)
                if hidx % 4 == 0 and ktp == max(1, 2 * qc + 1):
                    # V tiles for this window's diagonal k-tiles are
                    # needed by the PVs a couple of slots from now
                    drain_prep0(kind="v")
                run_slot_prelude()
                keep = 0 if (is_last and ktp >= n_pairs - 2) else pv_keep[0]
                run_prev_pv(keep=keep)
                while norm_q:
                    norm_q.pop(0)()

                norm = None
                if ktp == n_pairs - 1:
                    norm = (lambda oacc=oacc, hp=hp, h2=h2, qc=qc,
                            hidx=hidx: emit_norm(oacc, hp, h2, qc,
                                                 hidx, 0, 4))
                elif is_last and ktp == n_pairs - 2:
                    # early norm of subtiles 0,1: overlaps the tail;
                    # alternate DVE/Act so neither serializes the chains
                    def norm(oacc=oacc, hp=hp, h2=h2, qc=qc, hidx=hidx):
                        emit_norm(oacc, hp, h2, qc, hidx, 0, 1)
                        emit_norm(oacc, hp, h2, qc, hidx, 1, 2,
                                  use_act=True)
                pv_q.append(
                    (lambda pt=pt, ktp=ktp, oacc=oacc, h=h, qc=qc:
                     emit_pv(pt, ktp, oacc, h, qc), norm))

            # last head's final k-pair, split per-j: norms/transposes/
            # projections pipeline into the Exp gaps. Both j's S/Exp are
            # emitted first so ScalarE stays packed.
            hidx, (qc, hp, h2) = len(heads) - 1, heads[-1]
            h = 2 * hp + h2
            hb = 64 * h2
            q0 = qc * 512
            oacc = oaccs[hidx]
            ktp = 2 * (qc + 1) - 1
            pts = []
            for j in range(2):
                kt = 2 * ktp + j
                lo_j = kt * 128 - q0
                s = psA.tile([128, 2, 512], F32, tag="s", name=f"sL{j}")
                nc.tensor.matmul(
                    s[:, j, lo_j:W],
                    KT[hb:hb + 64, hp, kt * 128:(kt + 1) * 128],
                    QT[hb:hb + 64, hp, q0 + lo_j:q0 + W],
                    start=True, stop=True,
                )
                pt = ptp.tile([128, 2, 512], BF16, tag="pt", name=f"ptL{j}")
                nc.scalar.activation(pt[:, j, lo_j:W], s[:, j, lo_j:W],
                                     AF.Exp, scale=SCALE)
                nc.vector.tensor_mul(
                    pt[:, j, lo_j:lo_j + 128],
                    pt[:, j, lo_j:lo_j + 128],
                    tri[:, 0:128],
                )
                pts.append((pt, kt))
            run_prev_pv(keep=0)
            while norm_q:
                norm_q.pop(0)()
            final_tail[0] = True
            # both PV blocks + norms FIRST: the c3-critical chain must not
            # queue behind the earlier tiles' transposes/projections
            for j, (pt, kt) in enumerate(pts):
                for c in range(2 + j, 4):
                    nc.tensor.matmul(
                        oacc[:, c, :],
                        pt[:, j, c * 128:(c + 1) * 128],
                        Vaug[:, kt, h, :],
                        start=False, stop=(c == 2 + j),
                        skip_group_check=True,
                    )
                emit_norm(oacc, hp, h2, qc, hidx, 2 + j, 3 + j,
                          use_act=(j == 1))
            while aux_q:
                aux_q.pop(0)[1]()

            # drain anything left
            drain_all_prep()
            run_prev_pv()
            while norm_q or aux_q:
                if norm_q:
                    norm_q.pop(0)()
                else:
                    aux_q.pop(0)[1]()

    return nc


_NC_CACHE = None


def get_nc():
    global _NC_CACHE
    if _NC_CACHE is None:
        nc = bacc.Bacc("TRN2", target_bir_lowering=False, debug=False,
                       num_devices=NCORES)
        emit_core_program(nc)
        nc.compile()
        _NC_CACHE = nc
    return _NC_CACHE


def make_in_maps(x, w_qkv, w_out):
    import ml_dtypes
    BF = ml_dtypes.bfloat16
    x = np.asarray(x, dtype=np.float32)
    w_qkv = np.asarray(w_qkv, dtype=np.float32)
    w_out = np.asarray(w_out, dtype=np.float32)
    in_maps = []
    for c in range(NCORES):
        b, g = c // 2, c % 2
        lo = 256 * g
        in_maps.append({
            "xT": np.ascontiguousarray(x[b].T.astype(BF)),
            "wq": np.ascontiguousarray(w_qkv[:, lo:lo + 256].astype(BF)),
            "wk": np.ascontiguousarray(w_qkv[:, 512 + lo:512 + lo + 256].astype(BF)),
            "wv": np.ascontiguousarray(w_qkv[:, 1024 + lo:1024 + lo + 256].astype(BF)),
            "wo": np.ascontiguousarray(w_out[lo:lo + 256, :].astype(BF)),
        })
    return in_maps


def assemble_output(results):
    out = np.empty((B, T, D), dtype=np.float32)
    for b in range(B):
        out[b] = (np.asarray(results[2 * b]["y"], dtype=np.float32)
                  + np.asarray(results[2 * b + 1]["y"], dtype=np.float32))
    return out


def kernel(x, w_qkv, w_out):
    from concourse.bass_utils import run_bass_kernel_spmd

    nc = get_nc()
    in_maps = make_in_maps(x, w_qkv, w_out)
    res = run_bass_kernel_spmd(nc, in_maps, list(range(NCORES))).results
    return assemble_output(res)
